# revision 20
# baseline (speedup 1.0000x reference)
"""BotRGCN forward pass on 8 Trainium2 NeuronCores (Bass/Tile SPMD).

Strategy (graph/data parallel, per the sharding hint):
  - Nodes are split into 128-row blocks; core k owns blocks
    [k*bpc, (k+1)*bpc).
  - Feature encoder: x is transposed/cast to bf16 on host (xT [1552, npc]
    per core); one fused matmul against a combined block-sparse
    W_enc [1552,128], then the w_in linear.  The encoder emits h in both
    orientations (hT for later matmuls, row-major h for the gather table).
  - Per RGCN layer: AllGather the row-major bf16 h table so every core
    holds all N rows.  Edges are grouped by (super-block of SB=3 dst
    blocks, source shard of npc rows, relation); each group's source rows
    are fetched with batched dma_gather (InstDMAGatherAnt, <=1024 int16
    shard-relative indices per call, ~8x fewer Pool-engine ops than
    per-tile indirect DMA).  Per 128-edge tile a selection matrix
    S [128, 384] = (iota384 == block*128+dst_local) * (1/cnt) built on DVE
    (fp16: ints < 2048 exact) feeds a matmul accumulating
    P_r^T [feat, SB*128] per relation in PSUM.
    out^T = sum_r W_r^T P_r^T + root^T h^T (+bias, LeakyReLU) stays
    transposed; only the gather-table rows need an on-chip transpose.
    The classifier is fused into layer 2; the final output is produced
    transposed [128, npc] and transposed back on host.

kernel() takes FULL inputs and returns the FULL output.
"""

import math
import os as _os
import sys
from contextlib import ExitStack

sys.path.insert(0, "/opt/trn_rl_repo")

import ml_dtypes
import numpy as np

import concourse.bass as bass
import concourse.bacc as bacc_mod
import concourse.tile as tile
from concourse import mybir
from concourse.bass_utils import run_bass_kernel_spmd
from concourse.library_config import mlp as _mlp_lib
from concourse.masks import make_identity

NCORES = 8
P = 128
R = 2
SB = 3                   # dst blocks per super-block
GS = 6                   # super-blocks per gather group
QMAX = 8                 # tiles per dma_gather call (8*128 = 1024 idxs max)
D_IN = 1552
HID = 128
D_NUM, D_TWEET, D_CAT, D_DES = 5, 768, 11, 768

FLOW_DT = mybir.dt.float16       # fp16: 10-bit mantissa, ints <= 2048 exact
FLOW_NP = np.float16
S_DT = mybir.dt.float16

TRACE = False
LAST_RESULTS = None
TIME_RUNS = int(_os.environ.get("BOT_TIME_RUNS", "0"))
LAST_TIME_NS = None
LAST_TIMES = None

F32 = mybir.dt.float32
AF = mybir.ActivationFunctionType
ALU = mybir.AluOpType


def _host_prep(x, src, dst, et, weights):
    N = x.shape[0]
    E = src.shape[0]
    B_total = (N + P - 1) // P
    bpc = (B_total + NCORES - 1) // NCORES
    bpc = SB * ((bpc + SB - 1) // SB)               # multiple of SB
    npc = bpc * P
    Npad = npc * NCORES
    nsb = bpc // SB
    ngroups = (nsb + GS - 1) // GS

    # ---- combined encoder weight ----
    w_des, b_des = weights["w_des"], weights["b_des"]
    w_tweet, b_tweet = weights["w_tweet"], weights["b_tweet"]
    w_num, b_num = weights["w_num"], weights["b_num"]
    w_cat, b_cat = weights["w_cat"], weights["b_cat"]
    SUB = w_des.shape[1]
    W_enc = np.zeros((D_IN, 4 * SUB), np.float32)
    o_num, o_tweet, o_cat, o_des = 0, D_NUM, D_NUM + D_TWEET, D_NUM + D_TWEET + D_CAT
    W_enc[o_des:o_des + D_DES, 0 * SUB:1 * SUB] = w_des
    W_enc[o_tweet:o_tweet + D_TWEET, 1 * SUB:2 * SUB] = w_tweet
    W_enc[o_num:o_num + D_NUM, 2 * SUB:3 * SUB] = w_num
    W_enc[o_cat:o_cat + D_CAT, 3 * SUB:4 * SUB] = w_cat
    b_enc = np.concatenate([b_des, b_tweet, b_num, b_cat]).astype(np.float32)

    # ---- per-core x, pre-arranged so each 512-node chunk is one fully
    # contiguous [128, KCH*w] DMA: xq[p, KCH*off + k*w + i] = x[off+i, k*128+p]
    KCH = (D_IN + P - 1) // P
    SW = 512
    xTs = []
    for k in range(NCORES):
        lo, hi = k * npc, min((k + 1) * npc, N)
        xk = np.zeros((D_IN, npc), FLOW_NP)
        if hi > lo:
            xk[:, : hi - lo] = x[lo:hi].T.astype(FLOW_NP)
        xq = np.zeros((P, KCH * npc), FLOW_NP)
        off = 0
        while off < npc:
            w = min(SW, npc - off)
            for kc in range(KCH):
                pk = min(P, D_IN - kc * P)
                xq[:pk, KCH * off + kc * w: KCH * off + (kc + 1) * w] = \
                    xk[kc * P: kc * P + pk, off:off + w]
            off += w
        xTs.append(xq)

    # ---- edge plan ----
    scale = np.zeros(E, np.float32)
    for r in range(R):
        m = et == r
        cnt_r = np.bincount(dst[m], minlength=N)
        scale[m] = 1.0 / np.maximum(cnt_r[dst[m]], 1)

    gblk = dst >> 7                                  # global dst block
    kk = gblk // bpc                                 # owning core
    lb = gblk % bpc                                  # local block
    sbi = lb // SB                                   # super-block
    bis = lb % SB                                    # block within sb
    shard = src // npc                               # source shard
    idx16 = (src - shard * npc).astype(np.int64)     # shard-relative row

    # group id per (core, sb, shard, rel); count histogram
    ngrp = nsb * NCORES * NCORES * R                 # sb x core x shard x rel
    gid = ((kk * nsb + sbi) * NCORES + shard) * R + et
    cnt = np.bincount(gid, minlength=ngrp).reshape(NCORES, nsb, NCORES, R)

    # tiles per (sb, shard, rel): max over cores -> identical SPMD program
    Tg = np.maximum(1, (cnt.max(axis=0) + P - 1) // P)   # [nsb, NSH, R]

    # --- enumerate tiles in gather order: (g, s, sb in group, rel, t) ---
    # tile id -> used to index pdst/pscl; also per (g,s): column layout + calls
    tid_of = -np.ones((nsb, NCORES, R), np.int64)    # first tid of group
    col_of = -np.ones((nsb, NCORES, R), np.int64)    # first col in gb[s]
    ntiles_gs = np.zeros((ngroups, NCORES), np.int64)
    calls = [[[] for _ in range(NCORES)] for _ in range(ngroups)]
    TILES = 0
    for g in range(ngroups):
        sbs = range(g * GS, min((g + 1) * GS, nsb))
        for s in range(NCORES):
            col = 0
            for sb_ in sbs:
                for r in range(R):
                    tid_of[sb_, s, r] = TILES
                    col_of[sb_, s, r] = col
                    t = int(Tg[sb_, s, r])
                    TILES += t
                    col += t
            ntiles_gs[g, s] = col
            # split into calls of <= QMAX tiles
            off = 0
            while off < col:
                nt = min(QMAX, col - off)
                calls[g][s].append((off, nt))
                off += nt
    NTG_MAX = int(ntiles_gs.max())

    # ---- per-core slot assignment ----
    # rank of each edge within its (core, sb, shard, rel) group
    order = np.lexsort((idx16, gid))
    gs_sorted = gid[order]
    starts = np.zeros(ngrp + 1, np.int64)
    starts[1:] = np.cumsum(np.bincount(gs_sorted, minlength=ngrp))
    rank = np.empty(E, np.int64)
    rank[order] = np.arange(E) - starts[gs_sorted]

    tloc = rank >> 7                                 # tile within group
    pp_ = rank & 127                                 # partition
    tid = tid_of[sbi, shard, et] + tloc              # global tile id

    plan_idx16 = np.zeros((NCORES, P, TILES), np.int16)
    plan_pdst = np.full((NCORES, P, TILES), -1.0, np.float32)
    plan_pscl = np.zeros((NCORES, P, TILES), np.float32)
    plan_idx16[kk, pp_, tid] = idx16.astype(np.int16)
    plan_pdst[kk, pp_, tid] = (bis * P + (dst & 127)).astype(np.float32)
    plan_pscl[kk, pp_, tid] = scale

    # first tile id of each (group, shard) region; within a region tile ids
    # are consecutive in column order, so tid = tid0_gs[g, s] + col
    tid0_gs = np.zeros((ngroups, NCORES), np.int64)
    for g in range(ngroups):
        for s in range(NCORES):
            tid0_gs[g, s] = tid_of[g * GS, s, 0]

    # wrap idx into the dma_gather int16 layout: per call, slot j ->
    # partition j%16, column j//16; replicated across the 8 Q7 groups.
    IDXCOLS = TILES * 8                              # 128 idx / 16 per tile
    plan_idx = np.zeros((NCORES, P, IDXCOLS), np.int16)
    # global tile t occupies columns [t*8, (t+1)*8): slot j of the call is
    # (tile_in_call j>>7, partition j&127); within tile, slots are the 128
    # partitions in order, so col = t*8 + (p >> 4), row = p & 15.
    for k in range(NCORES):
        w = plan_idx16[k]                            # [128, TILES]
        wr = w.T.reshape(TILES, 8, 16)               # [t, p>>4, p&15]
        blk = np.transpose(wr, (2, 0, 1)).reshape(16, TILES * 8)
        plan_idx[k] = np.tile(blk, (8, 1))

    # consume lists: per (sb, rel): [(shard, col, tid), ...]
    consume = []
    for sb_ in range(nsb):
        per_rel = []
        for r in range(R):
            lst = []
            for s in range(NCORES):
                for t in range(int(Tg[sb_, s, r])):
                    lst.append((s, int(col_of[sb_, s, r]) + t,
                                int(tid_of[sb_, s, r]) + t))
            per_rel.append(lst)
        consume.append(per_rel)

    return dict(
        N=N, B_total=B_total, bpc=bpc, npc=npc, Npad=Npad, nsb=nsb,
        ngroups=ngroups,
        W_enc=W_enc, b_enc=b_enc, xTs=xTs,
        TILES=TILES, NTG_MAX=NTG_MAX, ntiles_gs=ntiles_gs, calls=calls,
        consume=consume, tid0_gs=tid0_gs,
        plan_idx=plan_idx, plan_pdst=plan_pdst, plan_pscl=plan_pscl,
    )


def _build_program(prep, weights):
    bpc, npc, Npad = prep["bpc"], prep["npc"], prep["Npad"]
    nsb, ngroups, TILES = prep["nsb"], prep["ngroups"], prep["TILES"]
    NTG_MAX, ntiles_gs, calls = prep["NTG_MAX"], prep["ntiles_gs"], prep["calls"]
    consume = prep["consume"]
    IDXCOLS = TILES * 8
    KCH = (D_IN + P - 1) // P                       # 13 chunks; last is 16 rows
    KFULL = D_IN // P                               # 12 full chunks

    nc = bacc_mod.Bacc(num_devices=NCORES, num_swdge_queues=4)

    # ---- I/O ----
    xT_t = nc.dram_tensor("xT", [P, KCH * npc], FLOW_DT, kind="ExternalInput")
    pidx_t = nc.dram_tensor("plan_idx", [P, IDXCOLS], mybir.dt.int16,
                            kind="ExternalInput")
    pdst_t = nc.dram_tensor("plan_pdst", [P, TILES], F32, kind="ExternalInput")
    pscl_t = nc.dram_tensor("plan_pscl", [P, TILES], F32, kind="ExternalInput")
    out_t = nc.dram_tensor("outT", [P, npc], F32, kind="ExternalOutput")

    # ---- internal DRAM ----
    h_rows = [nc.dram_tensor(f"h{l}_rows", [npc, HID], FLOW_DT) for l in range(2)]
    h_full = [nc.dram_tensor(f"h{l}_full", [Npad, HID], FLOW_DT, addr_space="Shared")
              for l in range(2)]
    hT = [nc.dram_tensor(f"h{l}T", [HID, npc], FLOW_DT) for l in range(2)]

    # ---- constants ----
    wenc_pad = np.zeros((KCH * P, HID), FLOW_NP)
    wenc_pad[:D_IN] = prep["W_enc"].astype(FLOW_NP)
    wenc_c = nc.inline_tensor(wenc_pad, "wenc")
    benc_c = nc.inline_tensor(prep["b_enc"].reshape(HID, 1), "benc")
    win_c = nc.inline_tensor(weights["w_in"].astype(FLOW_NP), "win")
    bin_c = nc.inline_tensor(
        weights["b_in"].astype(np.float32).reshape(HID, 1), "bin")

    lw = []
    for l, (wname, rname, bname) in enumerate(
        [("rg1_w", "rg1_root", "rg1_b"), ("rg2_w", "rg2_root", "rg2_b")]
    ):
        w = weights[wname].astype(FLOW_NP)
        root = weights[rname].astype(FLOW_NP)
        b = weights[bname].astype(np.float32).reshape(HID, 1)
        lw.append(dict(
            w0=nc.inline_tensor(w[0], f"l{l}w0"),
            w1=nc.inline_tensor(w[1], f"l{l}w1"),
            root=nc.inline_tensor(root, f"l{l}root"),
            b=nc.inline_tensor(b, f"l{l}b"),
        ))
    wcls_c = nc.inline_tensor(weights["w_cls"].astype(FLOW_NP), "wcls")
    bcls_c = nc.inline_tensor(
        weights["b_cls"].astype(np.float32).reshape(HID, 1), "bcls")
    iota3_c = nc.inline_tensor(
        np.tile(np.arange(SB * P, dtype=np.float32), (P, 1))
        .astype(np.float16), "iota3")

    with ExitStack() as ctx:
        tc = ctx.enter_context(tile.TileContext(
            nc, num_cores=NCORES, pool_alloc_mode="queue",
            trace_sim=bool(int(_os.environ.get("BOT_TRACE_SIM", "0")))))
        cp = ctx.enter_context(tc.tile_pool(name="const", bufs=1))

        wenc_sb = cp.tile([P, KCH * P], FLOW_DT)
        for k in range(KCH):
            pk = min(P, D_IN - k * P)
            nc.sync.dma_start(out=wenc_sb[:pk, k * P:(k + 1) * P],
                              in_=wenc_c[k * P:k * P + pk, :])
        benc_sb = cp.tile([P, 1], F32)
        nc.sync.dma_start(out=benc_sb[:], in_=benc_c[:, :])
        win_sb = cp.tile([P, P], FLOW_DT)
        nc.sync.dma_start(out=win_sb[:], in_=win_c[:, :])
        bin_sb = cp.tile([P, 1], F32)
        nc.sync.dma_start(out=bin_sb[:], in_=bin_c[:, :])
        lsb = []
        for l in range(2):
            d = {}
            for key in ("w0", "w1", "root"):
                t_ = cp.tile([P, P], FLOW_DT, tag=f"w_l{l}_{key}")
                nc.sync.dma_start(out=t_[:], in_=lw[l][key][:, :])
                d[key] = t_
            bt = cp.tile([P, 1], F32, tag=f"b_l{l}")
            nc.sync.dma_start(out=bt[:], in_=lw[l]["b"][:, :])
            d["b"] = bt
            lsb.append(d)
        wcls_sb = cp.tile([P, P], FLOW_DT)
        nc.sync.dma_start(out=wcls_sb[:], in_=wcls_c[:, :])
        bcls_sb = cp.tile([P, 1], F32)
        nc.sync.dma_start(out=bcls_sb[:], in_=bcls_c[:, :])
        iota3_sb = cp.tile([P, SB * P], S_DT)
        nc.sync.dma_start(out=iota3_sb[:], in_=iota3_c[:, :])
        ident_sb = cp.tile([P, P], FLOW_DT)
        make_identity(nc, ident_sb[:])

        pidx_sb = cp.tile([P, IDXCOLS], mybir.dt.int16)
        nc.sync.dma_start(out=pidx_sb[:], in_=pidx_t[:, :])
        pdst_sb = cp.tile([P, TILES], F32)
        nc.sync.dma_start(out=pdst_sb[:], in_=pdst_t[:, :])
        pscl_sb = cp.tile([P, TILES], F32)
        nc.sync.dma_start(out=pscl_sb[:], in_=pscl_t[:, :])

        nc.gpsimd.load_library(_mlp_lib)

        REPEAT = int(_os.environ.get("BOT_REPEAT", "1"))
        SKIP = set(_os.environ.get("BOT_SKIP", "").split(","))
        REPBAR = bool(int(_os.environ.get("BOT_REPBAR", "0")))
        AGBAR = bool(int(_os.environ.get("BOT_AGBAR", "0")))
        for _rep in range(REPEAT):
            if _rep > 0 and REPBAR:
                tc.strict_bb_all_engine_barrier()

            # ================= encoder =================
            with (
                tc.tile_pool(name="enc_sb", bufs=2) as ep,
                tc.tile_pool(name="enc_out", bufs=2) as hp_pool,
                tc.tile_pool(name="enc_rows", bufs=3) as rp,
                tc.tile_pool(name="enc_ps", bufs=2, space="PSUM") as pp,
            ):
                SW = 512
                off = 0
                while off < npc:
                    w = min(SW, npc - off)
                    nb = w // P
                    hpsum = pp.tile([P, SW], F32, tag="enc")
                    xall = ep.tile([P, KCH * SW], FLOW_DT, tag="xall")
                    nc.sync.dma_start(
                        out=xall[:, :KCH * w],
                        in_=xT_t[:, KCH * off:KCH * off + KCH * w])
                    for k in range(KCH):
                        pk = min(P, D_IN - k * P)
                        nc.tensor.matmul(out=hpsum[:, :w],
                                         lhsT=wenc_sb[:pk, k * P:(k + 1) * P],
                                         rhs=xall[:pk, k * w:(k + 1) * w],
                                         start=(k == 0), stop=(k == KCH - 1))
                    hs = hp_pool.tile([P, SW], FLOW_DT, tag="henc")
                    nc.scalar.activation(out=hs[:, :w], in_=hpsum[:, :w],
                                         func=AF.Lrelu, bias=benc_sb[:, :1],
                                         alpha=0.01)
                    h2psum = pp.tile([P, SW], F32, tag="enc2")
                    nc.tensor.matmul(out=h2psum[:, :w], lhsT=win_sb[:],
                                     rhs=hs[:, :w], start=True, stop=True)
                    hs2 = hp_pool.tile([P, SW], FLOW_DT, tag="henc2")
                    nc.scalar.activation(out=hs2[:, :w], in_=h2psum[:, :w],
                                         func=AF.Lrelu, bias=bin_sb[:, :1],
                                         alpha=0.01)
                    nc.sync.dma_start(out=hT[0][:, off:off + w], in_=hs2[:, :w])
                    rows = rp.tile([P, SW], FLOW_DT, tag="rows")
                    for t in range(nb):
                        tp = pp.tile([P, P], FLOW_DT, tag="tr")
                        nc.tensor.transpose(out=tp[:],
                                            in_=hs2[:, t * P:(t + 1) * P],
                                            identity=ident_sb[:])
                        nc.vector.tensor_copy(
                            out=rows[:, t * P:(t + 1) * P], in_=tp[:])
                    nc.sync.dma_start(
                        out=h_rows[0][off:off + w, :].rearrange(
                            "(b p) f -> p b f", p=P),
                        in_=rows[:, :w].rearrange("p (b f) -> p b f", f=P))
                    off += w

            if "ag" not in SKIP:
                nc.gpsimd.collective_compute(
                    "AllGather", ALU.bypass, replica_groups=[list(range(NCORES))],
                    ins=[h_rows[0][:, :]], outs=[h_full[0][:, :]])
            if AGBAR:
                tc.strict_bb_all_engine_barrier()

            # ================= RGCN layers =================
            for l in ([], range(2))["layers" not in SKIP]:
                with (
                    tc.tile_pool(name=f"l{l}_g", bufs=2) as gp,
                    tc.tile_pool(name=f"l{l}_s", bufs=8) as sp,
                    tc.tile_pool(name=f"l{l}_m", bufs=4) as mp,
                    tc.tile_pool(name=f"l{l}_ps", bufs=2, space="PSUM") as pp,
                ):
                    qrr = 0
                    for g in range(ngroups):
                        gbs = []
                        for s in range(NCORES):
                            gb = gp.tile([P, NTG_MAX * P], FLOW_DT,
                                         tag=f"gb{s}")
                            tid0 = int(prep["tid0_gs"][g, s])
                            for (toff, ntc) in calls[g][s]:
                                n_c = ntc * P
                                c0 = (tid0 + toff) * 8
                                if "gathers" not in SKIP:
                                    nc.gpsimd.dma_gather(
                                        out_ap=gb[:, toff * P:(toff + ntc) * P]
                                        .rearrange("p (t f) -> p t f", f=P),
                                        in_ap=h_full[l][s * npc:(s + 1) * npc, :],
                                        idxs_ap=pidx_sb[:, c0:c0 + ntc * 8],
                                        num_idxs=n_c,
                                        num_idxs_reg=n_c,
                                        elem_size=P,
                                        queue_num=qrr % 4,
                                    )
                                    qrr += 1
                                else:
                                    nc.sync.dma_start(
                                        out=gb[:, toff * P:(toff + ntc) * P]
                                        .rearrange("p (t f) -> p t f", f=P),
                                        in_=h_full[l][:n_c, :].rearrange(
                                            "(t p) f -> p t f", p=P))
                            gbs.append(gb)
                        for sb_ in range(g * GS, min((g + 1) * GS, nsb)):
                            hTb = mp.tile([P, SB * P], FLOW_DT, tag="hTb")
                            nc.sync.dma_start(
                                out=hTb[:],
                                in_=hT[l][:, sb_ * SB * P:(sb_ + 1) * SB * P])
                            Ps = mp.tile([P, R * SB * P], FLOW_DT, tag="ps")
                            for r in range(R):
                                tl = consume[sb_][r]
                                Pr = pp.tile([P, SB * P], F32, tag=f"P{r}")
                                for j, (s, col, ti) in enumerate(tl):
                                    S = sp.tile([P, SB * P], S_DT, tag="s")
                                    nc.vector.tensor_scalar(
                                        out=S[:], in0=iota3_sb[:],
                                        scalar1=pdst_sb[:, ti:ti + 1],
                                        scalar2=pscl_sb[:, ti:ti + 1],
                                        op0=ALU.is_equal, op1=ALU.mult)
                                    nc.tensor.matmul(
                                        out=Pr[:],
                                        lhsT=gbs[s][:, col * P:(col + 1) * P],
                                        rhs=S[:],
                                        start=(j == 0), stop=(j == len(tl) - 1))
                                nc.scalar.activation(
                                    out=Ps[:, r * SB * P:(r + 1) * SB * P],
                                    in_=Pr[:], func=AF.Copy)

                            op_ = pp.tile([P, SB * P], F32, tag="out")
                            for r in range(R):
                                nc.tensor.matmul(
                                    out=op_[:], lhsT=lsb[l][f"w{r}"][:],
                                    rhs=Ps[:, r * SB * P:(r + 1) * SB * P],
                                    start=(r == 0), stop=False)
                            nc.tensor.matmul(out=op_[:], lhsT=lsb[l]["root"][:],
                                             rhs=hTb[:], start=False, stop=True)
                            ho = mp.tile([P, SB * P], FLOW_DT, tag="ho")
                            nc.scalar.activation(out=ho[:], in_=op_[:],
                                                 func=AF.Lrelu,
                                                 bias=lsb[l]["b"][:, :1],
                                                 alpha=0.01)
                            if l == 0:
                                nc.sync.dma_start(
                                    out=hT[1][:, sb_ * SB * P:(sb_ + 1) * SB * P],
                                    in_=ho[:])
                                rows = mp.tile([P, SB * P], FLOW_DT, tag="rows")
                                for t in range(SB):
                                    tp = pp.tile([P, P], FLOW_DT, tag="tr2")
                                    nc.tensor.transpose(
                                        out=tp[:], in_=ho[:, t * P:(t + 1) * P],
                                        identity=ident_sb[:])
                                    nc.vector.tensor_copy(
                                        out=rows[:, t * P:(t + 1) * P], in_=tp[:])
                                nc.sync.dma_start(
                                    out=h_rows[1][sb_ * SB * P:(sb_ + 1) * SB * P,
                                                  :].rearrange(
                                        "(b p) f -> p b f", p=P),
                                    in_=rows[:, :].rearrange(
                                        "p (b f) -> p b f", f=P))
                            else:
                                cpsum = pp.tile([P, SB * P], F32, tag="tr2")
                                nc.tensor.matmul(out=cpsum[:], lhsT=wcls_sb[:],
                                                 rhs=ho[:], start=True, stop=True)
                                osb = mp.tile([P, SB * P], F32, tag="osb")
                                nc.vector.tensor_scalar(
                                    out=osb[:], in0=cpsum[:],
                                    scalar1=bcls_sb[:, :1],
                                    scalar2=None, op0=ALU.add)
                                nc.sync.dma_start(
                                    out=out_t[:, sb_ * SB * P:(sb_ + 1) * SB * P],
                                    in_=osb[:])

                if l == 0:
                    if "ag" not in SKIP:
                        nc.gpsimd.collective_compute(
                            "AllGather", ALU.bypass,
                            replica_groups=[list(range(NCORES))],
                            ins=[h_rows[1][:, :]], outs=[h_full[1][:, :]])
                    if AGBAR:
                        tc.strict_bb_all_engine_barrier()

    if not nc.is_finalized():
        nc.finalize()
    return nc


def kernel(**inputs):
    global LAST_RESULTS
    x = np.asarray(inputs["x"], np.float32)
    ei = np.asarray(inputs["edge_index"])
    et = np.asarray(inputs["edge_type"]).astype(np.int64)
    src = ei[0].astype(np.int64)
    dst = ei[1].astype(np.int64)

    weights = {k: np.asarray(v, np.float32) for k, v in inputs.items()
               if k not in ("x", "edge_index", "edge_type")}

    prep = _host_prep(x, src, dst, et, weights)
    nc = _build_program(prep, weights)

    in_maps = []
    for k in range(NCORES):
        in_maps.append({
            "xT": prep["xTs"][k],
            "plan_idx": prep["plan_idx"][k],
            "plan_pdst": prep["plan_pdst"][k],
            "plan_pscl": prep["plan_pscl"][k],
        })

    if TIME_RUNS > 0:
        results = _run_and_time(nc, in_maps, TIME_RUNS)
    else:
        res = run_bass_kernel_spmd(nc, in_maps, list(range(NCORES)), trace=TRACE)
        LAST_RESULTS = res
        results = res.results

    outs = [results[k]["outT"].T for k in range(NCORES)]
    out = np.concatenate(outs, axis=0)[: prep["N"]]
    return np.ascontiguousarray(out, dtype=np.float32)


def _run_and_time(nc, in_maps, n_runs):
    """Mirror bass2jax.run_bass_via_pjrt's multi-core path, but jit once,
    pre-place inputs on the device mesh, and wall-clock repeated executes."""
    global LAST_TIME_NS, LAST_TIMES
    import time as _time
    import jax
    from jax.sharding import Mesh, PartitionSpec, NamedSharding
    from jax.experimental.shard_map import shard_map
    from concourse import bass2jax, mybir as _mb
    bass2jax.install_neuronx_cc_hook()

    partition_name = nc.partition_id_tensor.name if nc.partition_id_tensor else None
    in_names, out_names, out_avals, zero_outs = [], [], [], []
    for alloc in nc.m.functions[0].allocations:
        if not isinstance(alloc, _mb.MemoryLocationSet):
            continue
        name = alloc.memorylocations[0].name
        if alloc.kind == "ExternalInput":
            if name != partition_name:
                in_names.append(name)
        elif alloc.kind == "ExternalOutput":
            shape = tuple(alloc.tensor_shape)
            dtype = _mb.dt.np(alloc.dtype)
            out_names.append(name)
            out_avals.append(jax.core.ShapedArray(shape, dtype))
            zero_outs.append(np.zeros(shape, dtype))
    n_params = len(in_names)
    in_names = in_names + out_names
    if partition_name is not None:
        in_names.append(partition_name)

    def _body(*args):
        operands = list(args)
        if partition_name is not None:
            operands.append(bass2jax.partition_id_tensor())
        outs = bass2jax._bass_exec_p.bind(
            *operands,
            out_avals=tuple(out_avals),
            in_names=tuple(in_names),
            out_names=tuple(out_names),
            lowering_input_output_aliases=(),
            sim_require_finite=True,
            sim_require_nnan=True,
            nc=nc,
        )
        return tuple(outs)

    devices = jax.devices()[:NCORES]
    mesh = Mesh(np.asarray(devices), ("core",))
    n_outs = len(out_names)
    in_specs = (PartitionSpec("core"),) * (n_params + n_outs)
    out_specs = (PartitionSpec("core"),) * n_outs
    sharded = jax.jit(
        shard_map(_body, mesh=mesh, in_specs=in_specs, out_specs=out_specs,
                  check_rep=False),
        keep_unused=True,
    )
    per_core = [[np.asarray(m[name]) for name in in_names[:n_params]]
                for m in in_maps]
    sh = NamedSharding(mesh, PartitionSpec("core"))
    concat_in = [
        jax.device_put(
            np.concatenate([per_core[c][i] for c in range(NCORES)], axis=0), sh)
        for i in range(n_params)
    ]
    concat_zeros = [
        jax.device_put(np.zeros((NCORES * z.shape[0], *z.shape[1:]), z.dtype), sh)
        for z in zero_outs
    ]
    jax.block_until_ready(concat_in)
    jax.block_until_ready(concat_zeros)

    times = []
    out_arrs = None
    for i in range(max(2, n_runs)):
        t0 = _time.perf_counter()
        out_arrs = sharded(*concat_in, *concat_zeros)
        jax.block_until_ready(out_arrs)
        times.append(_time.perf_counter() - t0)
    LAST_TIMES = times
    LAST_TIME_NS = int(min(times[1:]) * 1e9)
    return [
        {name: np.asarray(out_arrs[i]).reshape(NCORES, *out_avals[i].shape)[c]
         for i, name in enumerate(out_names)}
        for c in range(NCORES)
    ]
